# revision 18
# baseline (speedup 1.0000x reference)
"""
DLI loss kernel for Trainium2 (8 NeuronCores, pure data parallel over batch).

Math
----
The reference computes, per (b, j) window pair:
    logits[b,j,k] = h_last[b,j]@w_h + cterm[b,k] + fc_b
    loss_pair     = LSE_k(logits masked to k in [j+3, len_b)) - logits[b,j,j+3]
The h_last@w_h and fc_b terms are constant in k, so they cancel exactly
between the LSE and the positive logit.  The whole LSTM drops out and

    loss = sum_{b, s in [3, len_b)} [ log(sum_{k=s}^{len_b-1} e^{cterm[b,k]})
                                      - cterm[b,s] ] / sum_b (len_b - 3)
    cterm[b,k] = encoder_output[b,k,:] @ fc_w[0, H:]   (valid region only)

cterm values are O(+-2) so no max-subtraction is needed for a stable exp.

Device pipeline (per core, 16 batch rows)
-----------------------------------------
  - enc arrives through SWDGE (gpsimd) cast-DMAs that convert fp32 -> bf16
    in the DMA datapath: no on-chip cast pass.  2-row chunks; partition p
    holds rows (2b, 2b+1), t in {4p..4p+3} (2KB HBM lines).  The gpsimd
    queue carries ONLY these triggers, so descriptor generation starts at
    the top of the kernel.
  - All constants are precomputed on the host and DMA'd on the Scalar ring:
    identity (for PE transpose), the one-hot matvec weights
    woh[e, 16b+m] = w[e]*(m==b), and mask3 (f32 mask with first 3 cols
    zeroed).  The denominator sum(len_b - 3) is computed on the host.
  - PE: bf16 warm-up matmuls (HAM un-throttle), then per row 4 bf16
    transposes -> PSUM and a bf16 one-hot matvec (row b of cterm[16,512],
    PSUM-accumulated).  Transposes are LDWEIGHTS-bound (~107ns each).
  - PSUM->SBUF copies of transposed rows split across DVE and ACT; matvec
    for row b is emitted with skew 3 so it never head-blocks the PE queue.
  - Tail: exp (ACT; tables pre-warmed Ln-then-Exp so Exp is resident),
    masked suffix-sum scan (DVE), u=(S-1)*mask (DVE), a 1-element Ln that
    reads the exp output (cannot be hoisted) so the Ln table load overlaps
    the scan, Ln(x+1)+accumulate (ACT), masked-cterm accumulate (DVE).
  - Out = per-partition [16, 2] partials [ln_sum, mc_sum]; host computes
    numer = sum(c0 - c1) over partitions and cores, denom on host.
"""

import ml_dtypes
import numpy as np

import concourse.bacc as bacc
import concourse.bass as bass
import concourse.mybir as mybir
import concourse.tile as tile
from concourse._compat import with_exitstack
from concourse.bass_utils import run_bass_kernel_spmd

B, T, E, H = 128, 512, 128, 128
NCORES = 8
BPC = B // NCORES  # batch rows per core
NPAIR = BPC // 2

f32 = mybir.dt.float32
bf16 = mybir.dt.bfloat16

N_DUMMY = 8  # PE warm-up matmuls to lift the HAM clock gate


@with_exitstack
def _dli_body(ctx, tc):
    nc = tc.nc

    enc = nc.dram_tensor("enc", [BPC, T, E], f32, kind="ExternalInput").ap()
    mq = nc.dram_tensor("maskq", [4 * BPC, T // 4], f32, kind="ExternalInput").ap()
    mp = nc.dram_tensor("maskp", [4 * BPC, T // 4], f32, kind="ExternalInput").ap()
    lm = nc.dram_tensor("lmat", [4 * BPC, 4 * BPC], bf16, kind="ExternalInput").ap()
    woh_d = nc.dram_tensor("woh", [E, 32 * 32], bf16, kind="ExternalInput").ap()
    id_d = nc.dram_tensor("ident", [E, E], bf16, kind="ExternalInput").ap()
    out = nc.dram_tensor("out", [4 * BPC, 2], f32, kind="ExternalOutput").ap()

    const_pool = ctx.enter_context(tc.tile_pool(name="const", bufs=1))
    chunk_pool = ctx.enter_context(tc.tile_pool(name="chunk", bufs=NPAIR))
    t4_pool = ctx.enter_context(tc.tile_pool(name="t4", bufs=6))
    tp_psum = ctx.enter_context(tc.tile_pool(name="tp_psum", bufs=4, space="PSUM"))
    ct_psum = ctx.enter_context(tc.tile_pool(name="ct_psum", bufs=1, space="PSUM"))
    dm_psum = ctx.enter_context(tc.tile_pool(name="dm_psum", bufs=1, space="PSUM"))
    cy_psum = ctx.enter_context(tc.tile_pool(name="cy_psum", bufs=1, space="PSUM"))
    sc_pool = ctx.enter_context(tc.tile_pool(name="scan", bufs=1))

    # --- enc via SWDGE cast-DMAs (fp32 HBM -> bf16 SBUF), 2 rows per DMA.
    # These are the ONLY gpsimd-queue instructions, so generation starts
    # right after the engine prologue.
    groups = [(2 * p, 2) for p in range(NPAIR)]
    chunk_of = {}
    for lo, n in groups:
        chunk = chunk_pool.tile([128, n * T], bf16)
        for k in range(n):
            chunk_of[lo + k] = (chunk, k, n)
        nc.gpsimd.dma_start(
            chunk[:].rearrange("p (r c e) -> p r c e", r=n, c=4),
            enc[lo : lo + n].rearrange("r (a c) e -> a r c e", c=4),
        )

    # --- host-precomputed constants on the Scalar HWDGE ring ---
    ident = const_pool.tile([128, 128], bf16)
    nc.scalar.dma_start(ident[:], id_d)
    woh = const_pool.tile([128, 32 * 32], bf16)
    nc.scalar.dma_start(woh[:], woh_d)
    maskq = sc_pool.tile([4 * BPC, T // 4], f32)
    nc.scalar.dma_start(maskq[:], mq)
    maskp = sc_pool.tile([4 * BPC, T // 4], f32)
    nc.scalar.dma_start(maskp[:], mp)
    lmat = sc_pool.tile([4 * BPC, 4 * BPC], bf16)
    nc.scalar.dma_start(lmat[:], lm)

    # --- PE warm-up: real bf16 matmuls so HAM sees the PE busy ---
    ds = const_pool.tile([128, T], bf16)
    nc.vector.memset(ds[:].bitcast(mybir.dt.uint16), 0)
    dummy_ps = dm_psum.tile([128, T], f32)
    for _ in range(N_DUMMY):
        nc.tensor.matmul(
            dummy_ps[:, :], lhsT=ds[:, 0:128], rhs=ds[:], start=True, stop=True
        )

    # warm the ACT tables: Ln first, Exp last => the real Exp needs no table
    # load; the Ln reload is re-triggered right after the real Exp (below).
    warm = const_pool.tile([BPC, 1], f32)
    nc.vector.memset(warm[:], 0.0)
    nc.scalar.activation(warm[:], warm[:], mybir.ActivationFunctionType.Ln, bias=1.0)
    nc.scalar.activation(warm[:], warm[:], mybir.ActivationFunctionType.Exp)

    acc = sc_pool.tile([4 * BPC, 2], f32)

    # --- main loop: per row, 4 bf16 transposes + copy + 4 quarter matvecs.
    # cterm lives as [64, 128]: partition 4b + qr holds reversed-time quarter
    # qr of row b (qr=0 <-> t in [384,512)), so every tail op has a 128-long
    # free axis instead of 512.
    cterm_bank = ct_psum.tile([128, T], f32)  # full bank so nothing shares
    cterm_ps = cterm_bank[0 : 4 * BPC, 0 : T // 4]
    # matvec (b,q) writes 32-aligned partition blocks of cterm_ps; block h
    # accumulates across rows 8h..8h+7 (start/stop per 32-block)
    t4_tiles = [None] * BPC

    def emit_matvec(b):
        for q in range(4):
            m = 4 * b + (3 - q)
            h, k2 = divmod(m, 32)
            nc.tensor.matmul(
                cterm_ps[32 * h : 32 * (h + 1), :],
                lhsT=woh[:, 32 * k2 : 32 * (k2 + 1)],
                rhs=t4_tiles[b][:].rearrange("e (j p) -> e j p", j=4)[
                    :, :, 32 * q : 32 * (q + 1)
                ],
                start=(b % 8 == 0 and q == 0),
                stop=(b % 8 == 7 and q == 3),
                tile_position=(0, 32 * h),
            )

    copy_engines = [
        lambda o, i: nc.vector.tensor_copy(o, i),
        lambda o, i: nc.scalar.copy(o, i),
    ]
    # DVE gets 12 copies, ACT gets 4 (ACT also owns exp/ln + table loads)
    copy_sel = [0, 1, 0, 0, 1, 0, 0, 0, 0, 1, 0, 0, 1, 0, 0, 0]
    SKEW = 2
    for b in range(BPC):
        chunk, r, n = chunk_of[b]
        tp = tp_psum.tile([128, T], bf16)
        src = chunk[:].rearrange("p (r c e) -> p r c e", r=n, c=4)
        for j in range(4):
            nc.tensor.transpose(
                tp[:, 128 * j : 128 * (j + 1)], src[:, r, j], ident[:]
            )
        t4 = t4_pool.tile([128, T], bf16)
        t4_tiles[b] = t4
        copy_engines[copy_sel[b]](t4[:], tp[:])
        if b >= SKEW:
            emit_matvec(b - SKEW)
    for b in range(BPC - SKEW, BPC):
        emit_matvec(b)

    # E = exp(cterm): reversed within-quarter time via the AP pair.
    # cterm_ps[m, n] holds t_off = 4*(n%32) + n//32 of quarter 3-(m%4).
    e64 = sc_pool.tile([4 * BPC, T // 4], f32)
    nc.scalar.activation(
        e64[:].rearrange("m (p j) -> m p j", j=4),
        cterm_ps.rearrange("m (j p) -> m p j", j=4)[:, ::-1, ::-1],
        mybir.ActivationFunctionType.Exp,
    )
    # Re-trigger the Ln table load NOW so it overlaps the scan below.  Reads
    # e64 so the scheduler cannot hoist it before the exp.
    lnwarm = sc_pool.tile([1, 1], f32)
    nc.scalar.activation(
        lnwarm[:], e64[0:1, 0:1], mybir.ActivationFunctionType.Ln, bias=1.0
    )

    # per-quarter suffix sums with the mask folded in
    s64 = sc_pool.tile([4 * BPC, T // 4], f32)
    nc.vector.tensor_tensor_scan(
        s64[:], e64[:], maskq[:], 0.0, mybir.AluOpType.add, mybir.AluOpType.mult
    )
    # cross-quarter carries: carry[m] = sum of quarter totals of earlier
    # (later-t) quarters of the same row = Lmat^T @ totals.  A quarter with
    # leading invalid positions always has zero carry (its later-t quarters
    # are fully invalid), so the carry may be added unconditionally BEFORE
    # the mask multiply.
    carry_bank = cy_psum.tile([128, T], f32)
    carry_ps = carry_bank[0 : 4 * BPC, 0:1]
    # exact bf16 hi/lo split of the totals (fp32 lhsT matmuls are unreliable
    # on TRN2 hardware; lmat is 0/1 so bf16 weights are exact)
    tot_hi = sc_pool.tile([4 * BPC, 1], bf16)
    nc.vector.tensor_copy(tot_hi[:], s64[:, T // 4 - 1 :])
    tot_rem = sc_pool.tile([4 * BPC, 1], f32)
    nc.vector.tensor_tensor(
        tot_rem[:], s64[:, T // 4 - 1 :], tot_hi[:], mybir.AluOpType.subtract
    )
    tot_lo = sc_pool.tile([4 * BPC, 1], bf16)
    nc.vector.tensor_copy(tot_lo[:], tot_rem[:])
    nc.tensor.matmul(carry_ps, lhsT=lmat[:], rhs=tot_hi[:], start=True, stop=False)
    nc.tensor.matmul(carry_ps, lhsT=lmat[:], rhs=tot_lo[:], start=False, stop=True)
    cg_sb = sc_pool.tile([4 * BPC, 1], f32)
    nc.vector.tensor_copy(cg_sb[:], carry_ps)
    sfull = sc_pool.tile([4 * BPC, T // 4], f32)
    nc.vector.tensor_tensor(
        sfull[:], s64[:], cg_sb[:].broadcast_to([4 * BPC, T // 4]),
        mybir.AluOpType.add,
    )
    u64b = sc_pool.tile([4 * BPC, T // 4], f32)
    nc.vector.scalar_tensor_tensor(
        u64b[:], sfull[:], 1.0, maskq[:],
        mybir.AluOpType.subtract, mybir.AluOpType.mult,
    )
    ln64 = sc_pool.tile([4 * BPC, T // 4], f32)
    nc.scalar.activation(
        ln64[:], u64b[:], mybir.ActivationFunctionType.Ln,
        bias=1.0, scale=1.0, accum_out=acc[:, 0:1],
    )
    # sum(mask*cterm): both operands in the plain (j,p) quarter layout
    mc64 = sc_pool.tile([4 * BPC, T // 4], f32)
    nc.vector.scalar_tensor_tensor(
        mc64[:], cterm_ps, 0.0, maskp[:],
        mybir.AluOpType.add, mybir.AluOpType.mult, accum_out=acc[:, 1:2],
    )

    # out: per-partition partials; host computes sum(c0 - c1) / denom
    nc.sync.dma_start(out[:, :], acc[:])


_CACHED_NC = None


def _get_program():
    global _CACHED_NC
    if _CACHED_NC is None:
        nc = bacc.Bacc(
            "TRN2",
            target_bir_lowering=False,
            debug=False,
            enable_asserts=False,
        )
        with tile.TileContext(nc) as tc:
            _dli_body(tc)
        nc.compile()
        _CACHED_NC = nc
    return _CACHED_NC


def _make_in_maps(inputs):
    enc = np.ascontiguousarray(inputs["encoder_output"], dtype=np.float32)
    mask = np.asarray(inputs["mask"], dtype=np.int32)
    w_e = np.asarray(inputs["fc_w"], dtype=np.float32)[0, H:]

    mask3 = mask.astype(np.float32)
    mask3[:, 0:3] = 0.0

    # maskq[4b+qr, f] = mask3[b, 128*(3-qr) + 127 - f]  (reversed quarters)
    # maskp[4b+qr, n] = mask3[b, 128*(3-qr) + 4*(n%32) + n//32]  (plain)
    t_idx = np.arange(T).reshape(4, 128)          # [q, toff]
    maskq_full = np.zeros((B, 4, 128), dtype=np.float32)
    maskp_full = np.zeros((B, 4, 128), dtype=np.float32)
    n = np.arange(128)
    for qr in range(4):
        q = 3 - qr
        maskq_full[:, qr, :] = mask3[:, 128 * q + 127 - n]
        maskp_full[:, qr, :] = mask3[:, 128 * q + 4 * (n % 32) + n // 32]
    maskq_all = maskq_full.reshape(B * 4, 128)
    maskp_all = maskp_full.reshape(B * 4, 128)

    lmat = np.zeros((4 * BPC, 4 * BPC), dtype=np.float32)
    for p in range(4 * BPC):
        for m in range(4 * BPC):
            if p // 4 == m // 4 and p < m:
                lmat[p, m] = 1.0

    woh = np.zeros((E, 32 * 32), dtype=np.float32)
    for k2 in range(32):
        woh[:, 32 * k2 + k2] = w_e
    woh = woh.astype(ml_dtypes.bfloat16)
    ident = np.eye(E, dtype=ml_dtypes.bfloat16)

    return [
        {
            "enc": np.ascontiguousarray(enc[i * BPC : (i + 1) * BPC]),
            "maskq": np.ascontiguousarray(maskq_all[i * 4 * BPC : (i + 1) * 4 * BPC]),
            "maskp": np.ascontiguousarray(maskp_all[i * 4 * BPC : (i + 1) * 4 * BPC]),
            "lmat": lmat.astype(ml_dtypes.bfloat16),
            "woh": woh,
            "ident": ident,
        }
        for i in range(NCORES)
    ]


def _denoms(inputs):
    mask = np.asarray(inputs["mask"], dtype=np.int64)
    lengths = mask.sum(axis=1)
    return lengths - 3


def kernel(**inputs) -> np.ndarray:
    nc = _get_program()
    res = run_bass_kernel_spmd(nc, _make_in_maps(inputs), list(range(NCORES)))
    numer = 0.0
    for r in res.results:
        o = np.asarray(r["out"], dtype=np.float64)
        numer += float(np.sum(o[:, 0] - o[:, 1]))
    denom = float(np.sum(_denoms(inputs)))
    return np.asarray(numer / denom, dtype=np.float32)


# revision 20
# speedup vs baseline: 1.0165x; 1.0165x over previous
"""
DLI loss kernel for Trainium2 (8 NeuronCores, pure data parallel over batch).

Math
----
The reference computes, per (b, j) window pair:
    logits[b,j,k] = h_last[b,j]@w_h + cterm[b,k] + fc_b
    loss_pair     = LSE_k(logits masked to k in [j+3, len_b)) - logits[b,j,j+3]
The h_last@w_h and fc_b terms are constant in k, so they cancel exactly
between the LSE and the positive logit.  The whole LSTM drops out and

    loss = sum_{b, s in [3, len_b)} [ log(sum_{k=s}^{len_b-1} e^{cterm[b,k]})
                                      - cterm[b,s] ] / sum_b (len_b - 3)
    cterm[b,k] = encoder_output[b,k,:] @ fc_w[0, H:]   (valid region only)

cterm values are O(+-2) so no max-subtraction is needed for a stable exp.

Device pipeline (per core, 16 batch rows)
-----------------------------------------
  - enc arrives through SWDGE (gpsimd) cast-DMAs that convert fp32 -> bf16
    in the DMA datapath: no on-chip cast pass.  2-row chunks; partition p
    holds rows (2b, 2b+1), t in {4p..4p+3} (2KB HBM lines).  The gpsimd
    queue carries ONLY these triggers, so descriptor generation starts at
    the top of the kernel.
  - All constants are precomputed on the host and DMA'd on the Scalar ring:
    identity (for PE transpose), the one-hot matvec weights
    woh[e, 16b+m] = w[e]*(m==b), and mask3 (f32 mask with first 3 cols
    zeroed).  The denominator sum(len_b - 3) is computed on the host.
  - PE: bf16 warm-up matmuls (HAM un-throttle), then per row 4 bf16
    transposes -> PSUM and a bf16 one-hot matvec (row b of cterm[16,512],
    PSUM-accumulated).  Transposes are LDWEIGHTS-bound (~107ns each).
  - PSUM->SBUF copies of transposed rows split across DVE and ACT; matvec
    for row b is emitted with skew 3 so it never head-blocks the PE queue.
  - Tail: exp (ACT; tables pre-warmed Ln-then-Exp so Exp is resident),
    masked suffix-sum scan (DVE), u=(S-1)*mask (DVE), a 1-element Ln that
    reads the exp output (cannot be hoisted) so the Ln table load overlaps
    the scan, Ln(x+1)+accumulate (ACT), masked-cterm accumulate (DVE).
  - Out = per-partition [16, 2] partials [ln_sum, mc_sum]; host computes
    numer = sum(c0 - c1) over partitions and cores, denom on host.
"""

import ml_dtypes
import numpy as np

import concourse.bacc as bacc
import concourse.bass as bass
import concourse.mybir as mybir
import concourse.tile as tile
from concourse._compat import with_exitstack
from concourse.bass_utils import run_bass_kernel_spmd

B, T, E, H = 128, 512, 128, 128
NCORES = 8
BPC = B // NCORES  # batch rows per core
NPAIR = BPC // 2

f32 = mybir.dt.float32
bf16 = mybir.dt.bfloat16

N_DUMMY = 8  # PE warm-up matmuls to lift the HAM clock gate


@with_exitstack
def _dli_body(ctx, tc):
    nc = tc.nc

    enc = nc.dram_tensor("enc", [BPC, T, E], f32, kind="ExternalInput").ap()
    m3 = nc.dram_tensor("mask3", [BPC, T], f32, kind="ExternalInput").ap()
    woh_d = nc.dram_tensor("woh", [E, BPC * BPC], bf16, kind="ExternalInput").ap()
    id_d = nc.dram_tensor("ident", [E, E], bf16, kind="ExternalInput").ap()
    out = nc.dram_tensor("out", [BPC, 2], f32, kind="ExternalOutput").ap()

    const_pool = ctx.enter_context(tc.tile_pool(name="const", bufs=1))
    chunk_pool = ctx.enter_context(tc.tile_pool(name="chunk", bufs=NPAIR))
    t4_pool = ctx.enter_context(tc.tile_pool(name="t4", bufs=6))
    tp_psum = ctx.enter_context(tc.tile_pool(name="tp_psum", bufs=4, space="PSUM"))
    ct_psum = ctx.enter_context(tc.tile_pool(name="ct_psum", bufs=1, space="PSUM"))
    dm_psum = ctx.enter_context(tc.tile_pool(name="dm_psum", bufs=1, space="PSUM"))
    sc_pool = ctx.enter_context(tc.tile_pool(name="scan", bufs=1))

    # --- enc via SWDGE cast-DMAs (fp32 HBM -> bf16 SBUF), 2 rows per DMA.
    # These are the ONLY gpsimd-queue instructions, so generation starts
    # right after the engine prologue.
    groups = [(2 * p, 2) for p in range(NPAIR - 1)]
    chunk_of = {}
    for lo, n in groups:
        chunk = chunk_pool.tile([128, n * T], bf16)
        for k in range(n):
            chunk_of[lo + k] = (chunk, k, n)
        nc.gpsimd.dma_start(
            chunk[:].rearrange("p (r c e) -> p r c e", r=n, c=4),
            enc[lo : lo + n].rearrange("r (a c) e -> a r c e", c=4),
        )
    # rows 14-15 dodge the slow SWDGE path: fp32 via the idle Sync HWDGE
    # ring (starts ~2us earlier), cast to bf16 on DVE.  They are processed
    # FIRST, so the PE starts earlier and the tail pair is never gated by
    # the SWDGE straggler engine.
    chunk_f = chunk_pool.tile([128, 2 * T], f32)
    nc.sync.dma_start(
        chunk_f[:].rearrange("p (r c e) -> p r c e", r=2, c=4),
        enc[14:16].rearrange("r (a c) e -> a r c e", c=4),
    )
    chunk_e = chunk_pool.tile([128, 2 * T], bf16)
    chunk_of[14] = (chunk_e, 0, 2)
    chunk_of[15] = (chunk_e, 1, 2)

    # --- host-precomputed constants on the Scalar HWDGE ring ---
    ident = const_pool.tile([128, 128], bf16)
    nc.scalar.dma_start(ident[:], id_d)
    woh = const_pool.tile([128, BPC * BPC], bf16)
    nc.scalar.dma_start(woh[:], woh_d)
    maskf = sc_pool.tile([BPC, T], f32)
    nc.scalar.dma_start(maskf[:], m3)
    mask3_rev = maskf[:, ::-1]

    # --- PE warm-up: real bf16 matmuls so HAM sees the PE busy ---
    ds = const_pool.tile([128, T], bf16)
    nc.vector.memset(ds[:].bitcast(mybir.dt.uint16), 0)
    nc.vector.tensor_copy(chunk_e[:], chunk_f[:])
    dummy_ps = dm_psum.tile([128, T], f32)
    for _ in range(N_DUMMY):
        nc.tensor.matmul(
            dummy_ps[:, :], lhsT=ds[:, 0:128], rhs=ds[:], start=True, stop=True
        )

    # warm the ACT tables: Ln first, Exp last => the real Exp needs no table
    # load; the Ln reload is re-triggered right after the real Exp (below).
    warm = const_pool.tile([BPC, 1], f32)
    nc.vector.memset(warm[:], 0.0)
    nc.scalar.activation(warm[:], warm[:], mybir.ActivationFunctionType.Ln, bias=1.0)
    nc.scalar.activation(warm[:], warm[:], mybir.ActivationFunctionType.Exp)

    acc = sc_pool.tile([BPC, 2], f32)

    # --- main loop: per row, 4 bf16 transposes + copy + bf16 matvec ---
    cterm_ps = ct_psum.tile([BPC, T], f32)
    t4_tiles = [None] * BPC

    def emit_matvec(b):
        nc.tensor.matmul(
            cterm_ps[:, :],
            lhsT=woh[:, BPC * b : BPC * (b + 1)],
            rhs=t4_tiles[b][:],
            start=(b == 14),
            stop=(b == 13),
        )

    copy_engines = [
        lambda o, i: nc.vector.tensor_copy(o, i),
        lambda o, i: nc.scalar.copy(o, i),
    ]
    # DVE gets 12 copies, ACT gets 4 (ACT also owns exp/ln + table loads)
    copy_sel = [0, 1, 0, 0, 1, 0, 0, 0, 0, 1, 0, 0, 1, 0, 1, 0]
    row_order = [14, 15] + list(range(14))
    SKEW = 2
    for idx in range(BPC):
        b = row_order[idx]
        chunk, r, n = chunk_of[b]
        tp = tp_psum.tile([128, T], bf16)
        src = chunk[:].rearrange("p (r c e) -> p r c e", r=n, c=4)
        for j in range(4):
            nc.tensor.transpose(
                tp[:, 128 * j : 128 * (j + 1)], src[:, r, j], ident[:]
            )
        t4 = t4_pool.tile([128, T], bf16)
        t4_tiles[b] = t4
        copy_engines[copy_sel[b]](t4[:], tp[:])
        if idx >= SKEW:
            emit_matvec(row_order[idx - SKEW])
    for idx in range(BPC - SKEW, BPC):
        emit_matvec(row_order[idx])

    # un-permute + time-reverse view of the PSUM cterm: element i reads
    # cterm[b, 511 - i].
    cterm_rev = cterm_ps[:, :].rearrange("m (j p) -> m p j", j=4)[:, ::-1, ::-1]

    # E = exp(cterm)   (reversed-time coordinates, fused permute via the AP)
    e_sb = sc_pool.tile([BPC, T], f32)
    nc.scalar.activation(
        e_sb[:].rearrange("m (p j) -> m p j", j=4),
        cterm_rev,
        mybir.ActivationFunctionType.Exp,
    )
    # Re-trigger the Ln table load NOW so it overlaps the scan below.  Reads
    # e_sb so the scheduler cannot hoist it before the exp.
    lnwarm = sc_pool.tile([1, 1], f32)
    nc.scalar.activation(
        lnwarm[:], e_sb[0:1, 0:1], mybir.ActivationFunctionType.Ln, bias=1.0
    )

    # suffix sums with the mask folded in: state = (state + E[i]) * mask3_rev[i]
    s_sb = sc_pool.tile([BPC, T], f32)
    nc.vector.tensor_tensor_scan(
        s_sb[:], e_sb[:], mask3_rev, 0.0, mybir.AluOpType.add, mybir.AluOpType.mult
    )

    # u = (S - 1) * mask3; then ln(u + 1) = log(S) on valid, 0 on invalid
    u_sb = sc_pool.tile([BPC, T], f32)
    nc.vector.scalar_tensor_tensor(
        u_sb[:], s_sb[:], 1.0, mask3_rev,
        mybir.AluOpType.subtract, mybir.AluOpType.mult,
    )
    ln_sb = sc_pool.tile([BPC, T], f32)
    nc.scalar.activation(
        ln_sb[:], u_sb[:], mybir.ActivationFunctionType.Ln,
        bias=1.0, scale=1.0, accum_out=acc[:, 0:1],
    )
    # sum(mask3*cterm): order-free, so read the PSUM cterm unpermuted and the
    # mask through the matching permuted view.
    mc_sb = sc_pool.tile([BPC, T], f32)
    nc.vector.scalar_tensor_tensor(
        mc_sb[:].rearrange("m (j p) -> m j p", j=4),
        cterm_ps[:, :].rearrange("m (j p) -> m j p", j=4),
        0.0,
        maskf[:].rearrange("m (p j) -> m j p", j=4),
        mybir.AluOpType.add, mybir.AluOpType.mult, accum_out=acc[:, 1:2],
    )

    # out: per-partition partials; host computes sum(c0 - c1) / denom
    nc.sync.dma_start(out[:, :], acc[:])


_CACHED_NC = None


def _get_program():
    global _CACHED_NC
    if _CACHED_NC is None:
        nc = bacc.Bacc(
            "TRN2",
            target_bir_lowering=False,
            debug=False,
            enable_asserts=False,
        )
        with tile.TileContext(nc) as tc:
            _dli_body(tc)
        nc.compile()
        _CACHED_NC = nc
    return _CACHED_NC


def _make_in_maps(inputs):
    enc = np.ascontiguousarray(inputs["encoder_output"], dtype=np.float32)
    mask = np.asarray(inputs["mask"], dtype=np.int32)
    w_e = np.asarray(inputs["fc_w"], dtype=np.float32)[0, H:]

    mask3 = mask.astype(np.float32)
    mask3[:, 0:3] = 0.0

    woh = np.zeros((E, BPC * BPC), dtype=np.float32)
    for b in range(BPC):
        woh[:, BPC * b + b] = w_e
    woh = woh.astype(ml_dtypes.bfloat16)
    ident = np.eye(E, dtype=ml_dtypes.bfloat16)

    return [
        {
            "enc": np.ascontiguousarray(enc[i * BPC : (i + 1) * BPC]),
            "mask3": np.ascontiguousarray(mask3[i * BPC : (i + 1) * BPC]),
            "woh": woh,
            "ident": ident,
        }
        for i in range(NCORES)
    ]


def _denoms(inputs):
    mask = np.asarray(inputs["mask"], dtype=np.int64)
    lengths = mask.sum(axis=1)
    return lengths - 3


def kernel(**inputs) -> np.ndarray:
    nc = _get_program()
    res = run_bass_kernel_spmd(nc, _make_in_maps(inputs), list(range(NCORES)))
    numer = 0.0
    for r in res.results:
        o = np.asarray(r["out"], dtype=np.float64)
        numer += float(np.sum(o[:, 0] - o[:, 1]))
    denom = float(np.sum(_denoms(inputs)))
    return np.asarray(numer / denom, dtype=np.float32)
